# revision 1
# baseline (speedup 1.0000x reference)
"""Cross-modal attention Trainium2 kernel.

Sharding: 8 cores, one per (direction, batch, query-half):
  core = dir*4 + b*2 + qh
  dir 0: out1 rows (q from x1, k/v from x2); dir 1: out2 (q from x2, k/v from x1)
Each core computes a disjoint [1024, 512] slab of one output — no cross-core
reduction. All activations are kept transposed on device ([feature, token]),
so no on-device transposes are needed anywhere:
  qT/kT = W^T.T @ xT (per 128-feature chunk, heads pairwise stacked 64+64)
  scoresT[j,i] = k_j . q_i  (keys on partitions -> softmax denom comes free
  from an appended ones-column on v during the attn@v matmul)
  exp on ScalarE straight from PSUM at FD=1024, unnormalized attn@v into a
  PSUM accumulator per head, then per head: evacuate, reciprocal_approx of
  the denom row (DRAM-bounce partition-broadcast to base 0 first; DVE lanes
  are hard-wired to partitions and the custom recip uop is only correct at
  base partition 0), one multiply.
Scheduling: both heads of a pair emit score matmuls interleaved (base-0 and
base-64 row groups run concurrently in the PE array); attn@v is
software-pipelined one step behind exp; v projection and the next pair's q/k
projections are spread through the ACT-bound attention windows; input DMAs
split across the SP-HWDGE / ACT-HWDGE / SWDGE queues.
Biases: q/k folded into the PSUM->SBUF evacuation (per-partition adds);
v bias folded into the output-projection bias on the host (attn rows sum to 1);
1/sqrt(d) folded into Wq/bq on the host.
"""

import sys

sys.path.insert(0, "/opt/trn_rl_repo")

import numpy as np
import ml_dtypes

EMBED = 512
H = 8
D = 64
B = 2
L = 2048
LQ = 1024  # queries per core

_CACHE = {}


def _build_nc(reps=1):
    import concourse.bacc as bacc
    import concourse.mybir as mybir
    import concourse.tile as tile

    BF = mybir.dt.bfloat16
    F32 = mybir.dt.float32
    EXP = mybir.ActivationFunctionType.Exp

    nc = bacc.Bacc("TRN2", target_bir_lowering=False)

    # DRAM I/O (feature-chunked: [4, 128, N])
    xtq = nc.dram_tensor("xtq", [4, 128, LQ], BF, kind="ExternalInput")
    xtkv = nc.dram_tensor("xtkv", [4, 128, L], BF, kind="ExternalInput")
    wqt = nc.dram_tensor("wqt", [4, 128, 512], BF, kind="ExternalInput")
    wkt = nc.dram_tensor("wkt", [4, 128, 512], BF, kind="ExternalInput")
    wvt = nc.dram_tensor("wvt", [4, 128, 512], BF, kind="ExternalInput")
    wot = nc.dram_tensor("wot", [4, 128, 512], BF, kind="ExternalInput")
    bqd = nc.dram_tensor("bq", [4, 128, 1], F32, kind="ExternalInput")
    bkd = nc.dram_tensor("bk", [4, 128, 1], F32, kind="ExternalInput")
    bod = nc.dram_tensor("bo", [4, 128, 1], F32, kind="ExternalInput")
    yt = nc.dram_tensor("yt", [4, 128, LQ], F32, kind="ExternalOutput")

    with tile.TileContext(nc) as tc:
        with tc.tile_pool(name="persist", bufs=1) as persist:
            # ---- load inputs -------------------------------------------------
            xq_sb = [persist.tile([128, LQ], BF, name=f"xq{c}") for c in range(4)]
            xkv_sb = [persist.tile([128, L], BF, name=f"xkv{c}") for c in range(4)]
            wq_sb = [persist.tile([128, 512], BF, name=f"wq{c}") for c in range(4)]
            wk_sb = [persist.tile([128, 512], BF, name=f"wk{c}") for c in range(4)]
            wv_sb = [persist.tile([128, 512], BF, name=f"wv{c}") for c in range(4)]
            wo_sb = [persist.tile([128, 512], BF, name=f"wo{c}") for c in range(4)]
            bq_sb = [persist.tile([128, 1], F32, name=f"bq{c}") for c in range(4)]
            bk_sb = [persist.tile([128, 1], F32, name=f"bk{c}") for c in range(4)]
            bo_sb = [persist.tile([128, 1], F32, name=f"bo{c}") for c in range(4)]
            qt_sb = [persist.tile([128, LQ], BF, name=f"qt{f}") for f in range(4)]
            kt_sb = [persist.tile([128, L], BF, name=f"kt{f}") for f in range(4)]
            # v in natural layout, per 128-token chunk, heads strided by 65 so
            # each head slice [128, 65] carries its ones-column (softmax denom)
            v_sb = [persist.tile([128, H, D + 1], BF, name=f"v{l}") for l in range(16)]
            yat_sb = [persist.tile([128, LQ], BF, name=f"yat{f}") for f in range(4)]

            for _rep in range(reps):
                # q-path on the SP HWDGE ring, k-path on the ACT HWDGE
                # ring, v/out-path on SWDGE: three DMA streams in parallel so
                # the first score matmuls aren't gated on a serial load queue.
                for c in range(4):
                    nc.sync.dma_start(out=xq_sb[c], in_=xtq[c])
                    nc.sync.dma_start(out=wq_sb[c], in_=wqt[c])
                    nc.sync.dma_start(out=bq_sb[c], in_=bqd[c])
                for c in range(4):
                    nc.scalar.dma_start(out=xkv_sb[c], in_=xtkv[c])
                    nc.scalar.dma_start(out=wk_sb[c], in_=wkt[c])
                    nc.scalar.dma_start(out=bk_sb[c], in_=bkd[c])
                for c in range(4):
                    nc.gpsimd.dma_start(out=wv_sb[c], in_=wvt[c])
                    nc.gpsimd.dma_start(out=wo_sb[c], in_=wot[c])
                    nc.gpsimd.dma_start(out=bo_sb[c], in_=bod[c])

                for l in range(16):
                    nc.gpsimd.memset(v_sb[l], 1.0)

                with (
                    tc.tile_pool(name="scps", bufs=2, space="PSUM") as scps,
                    tc.tile_pool(name="avps", bufs=1, space="PSUM") as avps,
                    tc.tile_pool(name="att", bufs=6) as att,
                    tc.tile_pool(name="nrm", bufs=2) as nrm,
                    tc.tile_pool(name="dscr", bufs=2, space="DRAM") as dscr,
                ):
                    # prime the ScalarE exp table load during the DMA phase
                    dm = nrm.tile([1, 2], mybir.dt.float32, name="dm")
                    nc.vector.memset(dm, 0.0)
                    dm2 = nrm.tile([1, 2], mybir.dt.float32, name="dm2")
                    nc.scalar.activation(dm2, dm, EXP)
                    # warm the PE clock (HAM un-throttles after ~3.4us of
                    # sustained matmul activity) while input DMAs land
                    wup = nrm.tile([128, 512], BF, name="wup")
                    nc.vector.memset(wup, 0.0)
                    wps = scps.tile([128, 512], mybir.dt.float32, name="sc")
                    for i in range(20):
                        nc.tensor.matmul(
                            wps, wup[:, 0:128], wup, start=(i == 0), stop=(i == 19)
                        )

                    def qk_group(f, g):
                        # g 0..1: q i-halves; g 2..5: k quarters
                        ps = scps.tile([128, 512], mybir.dt.float32, name="sc")
                        if g < 2:
                            ih = g
                            for c in range(4):
                                nc.tensor.matmul(
                                    ps,
                                    wq_sb[c][:, f * 128 : (f + 1) * 128],
                                    xq_sb[c][:, ih * 512 : (ih + 1) * 512],
                                    start=(c == 0),
                                    stop=(c == 3),
                                )
                            nc.vector.tensor_scalar_add(
                                qt_sb[f][:, ih * 512 : (ih + 1) * 512], ps, bq_sb[f]
                            )
                        else:
                            ih = g - 2
                            for c in range(4):
                                nc.tensor.matmul(
                                    ps,
                                    wk_sb[c][:, f * 128 : (f + 1) * 128],
                                    xkv_sb[c][:, ih * 512 : (ih + 1) * 512],
                                    start=(c == 0),
                                    stop=(c == 3),
                                )
                            nc.vector.tensor_scalar_add(
                                kt_sb[f][:, ih * 512 : (ih + 1) * 512], ps, bk_sb[f]
                            )

                    def qk_proj(f):
                        for g in range(6):
                            qk_group(f, g)

                    def v_proj(l):
                        ps = scps.tile([128, 512], mybir.dt.float32, name="sc")
                        for c in range(4):
                            nc.tensor.matmul(
                                ps,
                                xkv_sb[c][:, l * 128 : (l + 1) * 128],
                                wv_sb[c],
                                start=(c == 0),
                                stop=(c == 3),
                            )
                        nc.vector.tensor_copy(
                            v_sb[l][:, :, 0:D], ps.rearrange("p (h d) -> p h d", h=H)
                        )

                    qk_proj(0)
                    v_proj(0)

                    for fc in range(4):  # head pair (2fc, 2fc+1)
                        av0 = avps.tile([65, LQ], mybir.dt.float32, name="av0")
                        av1 = avps.tile([65, LQ], mybir.dt.float32, name="av1")
                        avs = [av0, av1]
                        pend = None

                        def av_flush(p):
                            pex, pj = p
                            for hh in range(2):
                                for ih in range(2):
                                    nc.tensor.matmul(
                                        avs[hh][:, ih * 512 : (ih + 1) * 512],
                                        v_sb[pj][:, 2 * fc + hh, :],
                                        pex[hh][:, ih * 512 : (ih + 1) * 512],
                                        start=(pj == 0),
                                        stop=(pj == 15),
                                    )

                        for j in range(16):  # key chunks
                            # both heads' score matmuls interleaved: the 64-row
                            # groups (base 0 / base 64) run concurrently in PE
                            sc0 = scps.tile([128, LQ], mybir.dt.float32, name="sc")
                            sc1 = scps.tile([128, LQ], mybir.dt.float32, name="sc")
                            scs = [sc0, sc1]
                            for ih in range(2):
                                for hh in range(2):
                                    hp = hh * 64
                                    nc.tensor.matmul(
                                        scs[hh][:, ih * 512 : (ih + 1) * 512],
                                        kt_sb[fc][
                                            hp : hp + 64, j * 128 : (j + 1) * 128
                                        ],
                                        qt_sb[fc][
                                            hp : hp + 64, ih * 512 : (ih + 1) * 512
                                        ],
                                        start=True,
                                        stop=True,
                                    )
                            ex0 = att.tile([128, LQ], BF, name="ex0")
                            nc.scalar.activation(ex0, sc0, EXP)
                            ex1 = att.tile([128, LQ], BF, name="ex1")
                            nc.scalar.activation(ex1, sc1, EXP)
                            if fc == 0 and j + 1 < 16:
                                v_proj(j + 1)
                            if fc < 3 and 2 <= j < 14 and j % 2 == 0:
                                qk_group(fc + 1, (j - 2) // 2)
                            if pend is not None:
                                av_flush(pend)
                            pend = ([ex0, ex1], j)
                        av_flush(pend)

                        for hh in range(2):
                            # evacuate the accumulator at once (frees the PSUM
                            # slot early); normalize entirely at base 0 in SBUF
                            avc = nrm.tile([65, LQ], mybir.dt.float32, name="avc")
                            nc.vector.tensor_copy(avc, avs[hh])
                            dsc = dscr.tile([1, LQ], mybir.dt.float32, name="dsc")
                            nc.sync.dma_start(out=dsc, in_=avc[64:65, :])
                            rb = nrm.tile([64, LQ], mybir.dt.float32, name="rb")
                            nc.gpsimd.dma_start(out=rb, in_=dsc.to_broadcast([64, LQ]))
                            rbr = nrm.tile([64, LQ], mybir.dt.float32, name="rbr")
                            nc.vector.reciprocal_approx_fast(out=rbr, in_=rb)
                            nc.vector.tensor_mul(
                                yat_sb[fc][hh * 64 : hh * 64 + 64, :],
                                avc[0:64, :],
                                rbr,
                            )

                # ---- output projection ------------------------------------------
                with (
                    tc.tile_pool(name="ops", bufs=2, space="PSUM") as ops,
                    tc.tile_pool(name="yst", bufs=2) as yst,
                ):
                    for co in range(4):
                        yts = yst.tile([128, LQ], mybir.dt.float32)
                        for ih in range(2):
                            ps = ops.tile([128, 512], mybir.dt.float32)
                            for ci in range(4):
                                nc.tensor.matmul(
                                    ps,
                                    wo_sb[ci][:, co * 128 : (co + 1) * 128],
                                    yat_sb[ci][:, ih * 512 : (ih + 1) * 512],
                                    start=(ci == 0),
                                    stop=(ci == 3),
                                )
                            nc.vector.tensor_scalar_add(
                                yts[:, ih * 512 : (ih + 1) * 512], ps, bo_sb[co]
                            )
                        nc.sync.dma_start(out=yt[co], in_=yts)

    nc.finalize()
    return nc


def _prep_weights(qkv_w, qkv_b, out_w, out_b):
    bf = ml_dtypes.bfloat16
    w = qkv_w.reshape(H, 3, D, EMBED)
    b3 = qkv_b.reshape(H, 3, D)
    scale = 1.0 / np.sqrt(D).astype(np.float32)
    wq = w[:, 0].reshape(EMBED, EMBED) * scale
    wk = w[:, 1].reshape(EMBED, EMBED)
    wv = w[:, 2].reshape(EMBED, EMBED)
    bq = (b3[:, 0].reshape(EMBED) * scale).astype(np.float32)
    bk = b3[:, 1].reshape(EMBED).astype(np.float32)
    bv = b3[:, 2].reshape(EMBED).astype(np.float32)
    out = {
        "wqt": np.ascontiguousarray(wq.T).astype(bf).reshape(4, 128, 512),
        "wkt": np.ascontiguousarray(wk.T).astype(bf).reshape(4, 128, 512),
        "wvt": np.ascontiguousarray(wv.T).astype(bf).reshape(4, 128, 512),
        "wot": np.ascontiguousarray(out_w.T).astype(bf).reshape(4, 128, 512),
        "bq": bq.reshape(4, 128, 1),
        "bk": bk.reshape(4, 128, 1),
        "bo": (out_b + out_w @ bv).astype(np.float32).reshape(4, 128, 1),
    }
    return out


def kernel(x1, x2, qkv_w, qkv_b, out_w, out_b):
    from concourse.bass_utils import run_bass_kernel_spmd

    x1 = np.asarray(x1, dtype=np.float32)
    x2 = np.asarray(x2, dtype=np.float32)
    shared = _prep_weights(
        np.asarray(qkv_w, np.float32),
        np.asarray(qkv_b, np.float32),
        np.asarray(out_w, np.float32),
        np.asarray(out_b, np.float32),
    )

    bf = ml_dtypes.bfloat16
    xT = {
        0: [np.ascontiguousarray(x1[b].T).astype(bf) for b in range(B)],  # [512, L]
        1: [np.ascontiguousarray(x2[b].T).astype(bf) for b in range(B)],
    }

    in_maps = []
    for core in range(8):
        d, b, qh = core // 4, (core // 2) % 2, core % 2
        xq_mod = d  # dir 0 -> q from x1
        xkv_mod = 1 - d
        m = dict(shared)
        m["xtq"] = np.ascontiguousarray(
            xT[xq_mod][b][:, qh * LQ : (qh + 1) * LQ]
        ).reshape(4, 128, LQ)
        m["xtkv"] = xT[xkv_mod][b].reshape(4, 128, L)
        in_maps.append(m)

    if "nc" not in _CACHE:
        _CACHE["nc"] = _build_nc()
    try:
        res = run_bass_kernel_spmd(_CACHE["nc"], in_maps, core_ids=list(range(8)))
    except Exception:
        # transient runtime hiccups (e.g. a stale device state) recover on retry
        res = run_bass_kernel_spmd(_CACHE["nc"], in_maps, core_ids=list(range(8)))

    out1 = np.empty((B, L, EMBED), np.float32)
    out2 = np.empty((B, L, EMBED), np.float32)
    outs = {0: out1, 1: out2}
    for core in range(8):
        d, b, qh = core // 4, (core // 2) % 2, core % 2
        ytc = res.results[core]["yt"].reshape(512, LQ)
        outs[d][b, qh * LQ : (qh + 1) * LQ, :] = ytc.T
    return out1, out2

